# revision 25
# baseline (speedup 1.0000x reference)
"""Multi-head hierarchical attention Trainium2 kernel (8 NeuronCores).

Sharding: core c = (batch b, head-group g), b = c // 2, g = c % 2.
Each core computes, for one batch and 8 of the 16 heads:
  qh/kh/vh projections -> per-head softmax(QK^T * scale + cbias) @ V -> FC1
  partial (row-parallel over the head dim).  The host transposes activations/
  weights when building the per-core inputs (so the kernel needs no on-chip
  transposes), then sums the two FC1 partials per batch and adds the affine
  bias terms (b_fc1 and bv @ w_fc1.T, both exact affine folds).

Softmax skips the max-subtraction: scores = S/8 - 0.4 with |S| small by
construction, so exp() is computed directly and the denominator comes from an
extra ones-column in the PV matmul's stationary operand.
"""

import numpy as np

B = 4
L = 2048
D = 1024
N_HEADS = 16
DK = 64
N_GROUPS = 2                  # head groups (row-parallel FC1 shards)
H_LOC = N_HEADS // N_GROUPS   # 8 heads per core
DOUT = H_LOC * DK             # 512 projection dims per core
N_CORES = 8
SCALE = 0.125                 # 1/sqrt(DK)
CBIAS = 0.1 - 0.5             # BIAS_S + GAUSS_W


def build_module(seq_len=L):
    """Build + compile the Bass module for one core (SPMD; same for all)."""
    import concourse.bacc as bacc
    import concourse.tile as tile
    import concourse.mybir as mybir
    from concourse.bass import ts

    f32 = mybir.dt.float32
    f32r = mybir.dt.float32r
    bf16 = mybir.dt.bfloat16
    Exp = mybir.ActivationFunctionType.Exp

    TB = seq_len // 512       # 512-token blocks
    KT = seq_len // 128       # 128-token tiles
    MT = DOUT // 128          # output-dim tiles of the projections (4)
    KD = D // 128             # contraction tiles of the projections (8)
    NB = D // 512             # FC1 output blocks (2)
    KF = DOUT // 128          # FC1 contraction tiles (4)

    nc = bacc.Bacc("TRN2", target_bir_lowering=False)

    qT = nc.dram_tensor("qT", [D, seq_len], bf16, kind="ExternalInput")
    kT = nc.dram_tensor("kT", [D, seq_len], bf16, kind="ExternalInput")
    vT = nc.dram_tensor("vT", [D, seq_len], bf16, kind="ExternalInput")
    wqT = nc.dram_tensor("wqT", [D, DOUT], bf16, kind="ExternalInput")
    wkT = nc.dram_tensor("wkT", [D, DOUT], bf16, kind="ExternalInput")
    wvT = nc.dram_tensor("wvT", [D, DOUT], bf16, kind="ExternalInput")
    w1T = nc.dram_tensor("w1T", [DOUT, D], f32r, kind="ExternalInput")
    bq2 = nc.dram_tensor("bq2", [128, MT], f32, kind="ExternalInput")
    bk2 = nc.dram_tensor("bk2", [128, MT], f32, kind="ExternalInput")
    out = nc.dram_tensor("out", [seq_len, D], f32, kind="ExternalOutput")

    with tile.TileContext(nc) as tc:
        with (
            tc.tile_pool(name="persist", bufs=1) as persist,
            tc.tile_pool(name="persist2", bufs=1) as persist2,
            tc.tile_pool(name="inpool", bufs=10) as inpool,
        ):
            # Persistent: key heads [dout-part, t], value heads [t-part, h, d+1]
            # (ones column feeds the softmax denominator), q weights/bias.
            khT = [persist.tile([128, seq_len], bf16, tag=f"khT{m}", name=f"khT{m}") for m in range(MT)]
            vh = [persist.tile([128, H_LOC, DK + 1], bf16, tag=f"vh{t}", name=f"vh{t}") for t in range(KT)]
            qw = persist2.tile([128, KD, DOUT], bf16, tag="qw")
            qbias = persist2.tile([128, MT], f32, tag="qbias")
            ones64 = persist2.tile([1, 64], f32r, tag="ones64")
            cbias = persist2.tile([128, 1], f32, tag="cbias")

            for t in range(KT):
                nc.vector.memset(vh[t][:, :, :], 1.0)
            nc.vector.memset(ones64[:, :].bitcast(f32), 1.0)
            nc.vector.memset(cbias[:, :], CBIAS)

            nc.sync.dma_start(
                out=qw[:, :, :],
                in_=wqT[:, :].rearrange("(kk p) o -> p kk o", p=128),
            )
            nc.sync.dma_start(out=qbias[:, :], in_=bq2[:, :])

            # ---------------- Phase 1: K and V projections ----------------
            with (
                tc.tile_pool(name="p1w", bufs=2) as wpool,
                tc.tile_pool(name="p1b", bufs=1) as bpool,
                tc.tile_pool(name="p1ps", bufs=4, space="PSUM") as pspool,
            ):
                for xdram, wdram, bdram, kind in (
                    (vT, wvT, None, "v"),
                    (kT, wkT, bk2, "k"),
                ):
                    wsb = wpool.tile([128, KD, DOUT], bf16, tag="w")
                    nc.sync.dma_start(
                        out=wsb[:, :, :],
                        in_=wdram[:, :].rearrange("(kk p) o -> p kk o", p=128),
                    )
                    if bdram is not None:
                        bsb = bpool.tile([128, MT], f32, tag="b")
                        nc.sync.dma_start(out=bsb[:, :], in_=bdram[:, :])
                    for tb in range(TB):
                        xin = []
                        for kk in range(KD):
                            xt = inpool.tile([128, 512], bf16, tag="xin")
                            nc.sync.dma_start(
                                out=xt[:, :],
                                in_=xdram[ts(kk, 128), ts(tb, 512)],
                            )
                            xin.append(xt)
                        if kind == "k":
                            for m in range(MT):
                                ps = pspool.tile([128, 512], f32, tag="ps")
                                for kk in range(KD):
                                    nc.tensor.matmul(
                                        ps[:, :],
                                        lhsT=wsb[:, kk, ts(m, 128)],
                                        rhs=xin[kk][:, :],
                                        start=(kk == 0),
                                        stop=(kk == KD - 1),
                                    )
                                nc.vector.tensor_scalar_add(
                                    khT[m][:, ts(tb, 512)], ps[:, :],
                                    bsb[:, m : m + 1],
                                )
                        else:
                            for tt in range(4):
                                ps = pspool.tile([128, 512], f32, tag="ps")
                                for kk in range(KD):
                                    nc.tensor.matmul(
                                        ps[:, :],
                                        lhsT=xin[kk][:, ts(tt, 128)],
                                        rhs=wsb[:, kk, :],
                                        start=(kk == 0),
                                        stop=(kk == KD - 1),
                                    )
                                vt = vh[tb * 4 + tt]
                                nc.vector.tensor_copy(
                                    vt[:, :, 0:DK],
                                    ps[:, :].rearrange("p (h d) -> p h d", d=DK),
                                )

            # ------ Phase 2: q-proj + attention + FC1, fused per q-block ---
            with (
                tc.tile_pool(name="p2s", bufs=2, space="PSUM") as spool,
                tc.tile_pool(name="p2pv", bufs=2, space="PSUM") as pvpool,
                tc.tile_pool(name="p2x", bufs=2, space="PSUM") as xpool,
                tc.tile_pool(name="p2e", bufs=8) as epool,
                tc.tile_pool(name="p2r", bufs=2) as rpool,
                tc.tile_pool(name="p2c", bufs=2) as cpool,
                tc.tile_pool(name="p2q", bufs=2) as qpool,
                tc.tile_pool(name="p3w", bufs=1) as w1pool,
                tc.tile_pool(name="p3o", bufs=2) as opool,
            ):
                w1sb = w1pool.tile([128, KF, D], f32r, tag="w1")
                nc.sync.dma_start(
                    out=w1sb[:, :, :],
                    in_=w1T[:, :].rearrange("(kt p) o -> p kt o", p=128),
                )

                def qproj_m(qb, qh, xin, m):
                    ps = xpool.tile([128, 512], f32, tag="x", name="qps")
                    for kk in range(KD):
                        nc.tensor.matmul(
                            ps[:, :],
                            lhsT=qw[:, kk, ts(m, 128)],
                            rhs=xin[kk][:, :],
                            start=(kk == 0),
                            stop=(kk == KD - 1),
                        )
                    nc.vector.tensor_scalar_add(
                        qh[m][:, :], ps[:, :], qbias[:, m : m + 1]
                    )

                def q_load(qb):
                    qh = [
                        qpool.tile([128, 512], bf16, tag=f"qh{m}", name=f"qh{m}")
                        for m in range(MT)
                    ]
                    xin = []
                    for kk in range(KD):
                        xt = inpool.tile([128, 512], bf16, tag="xin")
                        nc.sync.dma_start(
                            out=xt[:, :], in_=qT[ts(kk, 128), ts(qb, 512)]
                        )
                        xin.append(xt)
                    return qh, xin

                def attn_chunk(qh, h, pv, ktp):
                    hp, h01 = divmod(h, 2)
                    po = h01 * 64
                    sps = spool.tile([128, 1024], f32, tag="s", name="sps")
                    for j in range(2):
                        kt = 2 * ktp + j
                        nc.tensor.matmul(
                            sps[:, ts(j, 512)],
                            lhsT=khT[hp][po : po + 64, ts(kt, 128)],
                            rhs=qh[hp][po : po + 64, :],
                            start=True,
                            stop=True,
                        )
                    e = epool.tile([128, 1024], bf16, tag="e", name="e")
                    nc.scalar.activation(
                        e[:, :], sps[:, :], Exp,
                        bias=cbias[:, 0:1], scale=SCALE,
                    )
                    for j in range(2):
                        kt = 2 * ktp + j
                        nc.tensor.matmul(
                            pv[:, :],
                            lhsT=vh[kt][:, h, :],
                            rhs=e[:, ts(j, 512)],
                            start=(kt == 0),
                            stop=(kt == KT - 1),
                        )

                def attn_norm(ctx, h, pv):
                    hp, h01 = divmod(h, 2)
                    po = h01 * 64
                    za = rpool.tile([1, 512], f32r, tag="za", name="za")
                    nc.vector.tensor_copy(za[0:1, :], pv[64:65, :])
                    zbps = xpool.tile([128, 512], f32, tag="x", name="zbps")
                    nc.tensor.matmul(
                        zbps[0:64, :], lhsT=ones64[:, :], rhs=za[:, :],
                        start=True, stop=True,
                    )
                    rrec = rpool.tile([64, 512], f32, tag="rrec", name="rrec")
                    nc.vector.reciprocal(rrec[:, :], zbps[0:64, :])
                    nc.vector.tensor_mul(
                        ctx[hp][po : po + 64, :], pv[0:64, :], rrec[0:64, :]
                    )

                def fc1_tq(ctx, tq, qt):
                    osb = opool.tile([128, D], f32, tag="o")
                    for nb in range(NB):
                        fps = xpool.tile([128, 512], f32, tag="x", name="fps")
                        for kt4 in range(KF):
                            nc.tensor.matmul(
                                fps[:, :],
                                lhsT=ctx[kt4][:, ts(tq, 128)],
                                rhs=w1sb[:, kt4, ts(nb, 512)],
                                start=(kt4 == 0),
                                stop=(kt4 == KF - 1),
                            )
                        nc.vector.tensor_copy(osb[:, ts(nb, 512)], fps[:, :])
                    nc.sync.dma_start(out=out[ts(qt, 128), :], in_=osb[:, :])

                prev = None     # (ctx, qb) of the previous block, for FC1
                pending = None  # deferred normalization of the last head
                qh, xin = q_load(0)
                for m in range(MT):
                    qproj_m(0, qh, xin, m)
                for qb in range(TB):
                    ctx = [
                        cpool.tile([128, 512], f32r, tag=f"ctx{hp}", name=f"ctx{hp}")
                        for hp in range(MT)
                    ]
                    if qb + 1 < TB:
                        qh_next, xin_next = q_load(qb + 1)
                    for h in range(H_LOC):
                        pv = pvpool.tile([65, 512], f32, tag="pv", name="pv")
                        attn_chunk(qh, h, pv, 0)
                        # deferred work lands while exp(h,0) runs on ScalarE
                        if pending is not None:
                            attn_norm(*pending)
                        if h % 2 == 0:
                            if prev is not None:
                                fc1_tq(prev[0], h // 2, prev[1] * 4 + h // 2)
                        else:
                            if qb + 1 < TB:
                                qproj_m(qb + 1, qh_next, xin_next, h // 2)
                        for ktp in range(1, KT // 2):
                            attn_chunk(qh, h, pv, ktp)
                        pending = (ctx, h, pv)
                    prev = (ctx, qb)
                    if qb + 1 < TB:
                        qh, xin = qh_next, xin_next
                attn_norm(*pending)
                for tq in range(4):
                    fc1_tq(prev[0], tq, prev[1] * 4 + tq)

    nc.compile()
    return nc


_module_cache = {}


def _get_module(seq_len=L):
    if seq_len not in _module_cache:
        _module_cache[seq_len] = build_module(seq_len)
    return _module_cache[seq_len]


def make_in_maps(q, k_s, v_s, wq, bq, wk, bk, wv, bv, w_fc1, b_fc1):
    """Host-side sharding: per-core input dict (transposed layouts)."""
    import ml_dtypes
    bf = ml_dtypes.bfloat16
    in_maps = []
    for c in range(N_CORES):
        b, g = divmod(c, N_GROUPS)
        sl = slice(g * DOUT, (g + 1) * DOUT)
        in_maps.append(
            {
                "qT": np.ascontiguousarray(q[b].T).astype(bf),
                "kT": np.ascontiguousarray(k_s[b].T).astype(bf),
                "vT": np.ascontiguousarray(v_s[b].T).astype(bf),
                "wqT": np.ascontiguousarray(wq[sl, :].T).astype(bf),
                "wkT": np.ascontiguousarray(wk[sl, :].T).astype(bf),
                "wvT": np.ascontiguousarray(wv[sl, :].T).astype(bf),
                "w1T": np.ascontiguousarray(w_fc1[:, sl].T),
                "bq2": np.ascontiguousarray(bq[sl].reshape(DOUT // 128, 128).T),
                "bk2": np.ascontiguousarray(bk[sl].reshape(DOUT // 128, 128).T),
            }
        )
    return in_maps


def kernel(q, k_w, v_w, k_s, v_s, wq, bq, wk, bk, wv, bv, w_fc1, b_fc1):
    from concourse.bass_utils import run_bass_kernel_spmd

    q = np.asarray(q, np.float32)
    k_s = np.asarray(k_s, np.float32)
    v_s = np.asarray(v_s, np.float32)
    wq = np.asarray(wq, np.float32)
    bq = np.asarray(bq, np.float32)
    wk = np.asarray(wk, np.float32)
    bk = np.asarray(bk, np.float32)
    wv = np.asarray(wv, np.float32)
    bv = np.asarray(bv, np.float32)
    w_fc1 = np.asarray(w_fc1, np.float32)
    b_fc1 = np.asarray(b_fc1, np.float32)

    nc = _get_module(L)
    in_maps = make_in_maps(q, k_s, v_s, wq, bq, wk, bk, wv, bv, w_fc1, b_fc1)
    res = run_bass_kernel_spmd(nc, in_maps, core_ids=list(range(N_CORES)))

    # Gather: sum the two row-parallel FC1 partials per batch, then add the
    # affine terms folded out of the device kernel (exact):
    #   ctx = P@V + bv  =>  out += bv @ w_fc1.T ; plus b_fc1.
    const = (b_fc1 + bv @ w_fc1.T).astype(np.float32)
    out = np.empty((B, L, D), np.float32)
    for b in range(B):
        out[b] = (
            res.results[N_GROUPS * b]["out"]
            + res.results[N_GROUPS * b + 1]["out"]
            + const
        )
    return out


# revision 26
# speedup vs baseline: 1.3022x; 1.3022x over previous
"""Multi-head hierarchical attention Trainium2 kernel (8 NeuronCores).

Sharding: core c = (batch b, head-group g), b = c // 2, g = c % 2.
Each core computes, for one batch and 8 of the 16 heads:
  qh/kh/vh projections -> per-head softmax(QK^T * scale + cbias) @ V -> FC1
  partial (row-parallel over the head dim).  The host transposes activations/
  weights when building the per-core inputs (so the kernel needs no on-chip
  transposes), then sums the two FC1 partials per batch and adds the affine
  bias terms (b_fc1 and bv @ w_fc1.T, both exact affine folds).

Softmax skips the max-subtraction: scores = S/8 - 0.4 with |S| small by
construction, so exp() is computed directly and the denominator comes from an
extra ones-column in the PV matmul's stationary operand.
"""

import numpy as np

B = 4
L = 2048
D = 1024
N_HEADS = 16
DK = 64
N_GROUPS = 2                  # head groups (row-parallel FC1 shards)
H_LOC = N_HEADS // N_GROUPS   # 8 heads per core
DOUT = H_LOC * DK             # 512 projection dims per core
N_CORES = 8
SCALE = 0.125                 # 1/sqrt(DK)
CBIAS = 0.1 - 0.5             # BIAS_S + GAUSS_W


def build_module(seq_len=L):
    """Build + compile the Bass module for one core (SPMD; same for all)."""
    import concourse.bacc as bacc
    import concourse.tile as tile
    import concourse.mybir as mybir
    from concourse.bass import ts

    f32 = mybir.dt.float32
    f32r = mybir.dt.float32r
    bf16 = mybir.dt.bfloat16
    Exp = mybir.ActivationFunctionType.Exp

    TB = seq_len // 512       # 512-token blocks
    KT = seq_len // 128       # 128-token tiles
    MT = DOUT // 128          # output-dim tiles of the projections (4)
    KD = D // 128             # contraction tiles of the projections (8)
    NB = D // 512             # FC1 output blocks (2)
    KF = DOUT // 128          # FC1 contraction tiles (4)

    nc = bacc.Bacc("TRN2", target_bir_lowering=False)

    qT = nc.dram_tensor("qT", [D, seq_len], bf16, kind="ExternalInput")
    kT = nc.dram_tensor("kT", [D, seq_len], bf16, kind="ExternalInput")
    vT = nc.dram_tensor("vT", [D, seq_len], bf16, kind="ExternalInput")
    wqT = nc.dram_tensor("wqT", [D, DOUT], bf16, kind="ExternalInput")
    wkT = nc.dram_tensor("wkT", [D, DOUT], bf16, kind="ExternalInput")
    wvT = nc.dram_tensor("wvT", [D, DOUT], bf16, kind="ExternalInput")
    w1T = nc.dram_tensor("w1T", [DOUT, D], f32r, kind="ExternalInput")
    bq2 = nc.dram_tensor("bq2", [128, MT], f32, kind="ExternalInput")
    bk2 = nc.dram_tensor("bk2", [128, MT], f32, kind="ExternalInput")
    out = nc.dram_tensor("out", [seq_len, D], f32, kind="ExternalOutput")

    with tile.TileContext(nc) as tc:
        with (
            tc.tile_pool(name="persist", bufs=1) as persist,
            tc.tile_pool(name="persist2", bufs=1) as persist2,
            tc.tile_pool(name="inpool", bufs=10) as inpool,
        ):
            # Persistent: key heads [dout-part, t], value heads [t-part, h, d+1]
            # (ones column feeds the softmax denominator), q weights/bias.
            khT = [persist.tile([128, seq_len], bf16, tag=f"khT{m}", name=f"khT{m}") for m in range(MT)]
            vh = [persist.tile([128, H_LOC, DK + 1], bf16, tag=f"vh{t}", name=f"vh{t}") for t in range(KT)]
            qw = persist2.tile([128, KD, DOUT], bf16, tag="qw")
            qbias = persist2.tile([128, MT], f32, tag="qbias")
            ones64 = persist2.tile([1, 64], f32r, tag="ones64")
            cbias = persist2.tile([128, 1], f32, tag="cbias")

            for t in range(KT):
                nc.vector.memset(vh[t][:, :, :], 1.0)
            nc.vector.memset(ones64[:, :].bitcast(f32), 1.0)
            nc.vector.memset(cbias[:, :], CBIAS)

            nc.sync.dma_start(
                out=qw[:, :, :],
                in_=wqT[:, :].rearrange("(kk p) o -> p kk o", p=128),
            )
            nc.sync.dma_start(out=qbias[:, :], in_=bq2[:, :])

            # ---------------- Phase 1: K and V projections ----------------
            with (
                tc.tile_pool(name="p1w", bufs=2) as wpool,
                tc.tile_pool(name="p1b", bufs=1) as bpool,
                tc.tile_pool(name="p1ps", bufs=4, space="PSUM") as pspool,
            ):
                for xdram, wdram, bdram, kind in (
                    (vT, wvT, None, "v"),
                    (kT, wkT, bk2, "k"),
                ):
                    wsb = wpool.tile([128, KD, DOUT], bf16, tag="w")
                    nc.sync.dma_start(
                        out=wsb[:, :, :],
                        in_=wdram[:, :].rearrange("(kk p) o -> p kk o", p=128),
                    )
                    if bdram is not None:
                        bsb = bpool.tile([128, MT], f32, tag="b")
                        nc.sync.dma_start(out=bsb[:, :], in_=bdram[:, :])
                    for tb in range(TB):
                        xin = []
                        for kk in range(KD):
                            xt = inpool.tile([128, 512], bf16, tag="xin")
                            nc.sync.dma_start(
                                out=xt[:, :],
                                in_=xdram[ts(kk, 128), ts(tb, 512)],
                            )
                            xin.append(xt)
                        if kind == "k":
                            for m in range(MT):
                                ps = pspool.tile([128, 512], f32, tag="ps")
                                for kk in range(KD):
                                    nc.tensor.matmul(
                                        ps[:, :],
                                        lhsT=wsb[:, kk, ts(m, 128)],
                                        rhs=xin[kk][:, :],
                                        start=(kk == 0),
                                        stop=(kk == KD - 1),
                                    )
                                nc.vector.tensor_scalar_add(
                                    khT[m][:, ts(tb, 512)], ps[:, :],
                                    bsb[:, m : m + 1],
                                )
                        else:
                            for tt in range(4):
                                ps = pspool.tile([128, 512], f32, tag="ps")
                                for kk in range(KD):
                                    nc.tensor.matmul(
                                        ps[:, :],
                                        lhsT=xin[kk][:, ts(tt, 128)],
                                        rhs=wsb[:, kk, :],
                                        start=(kk == 0),
                                        stop=(kk == KD - 1),
                                    )
                                vt = vh[tb * 4 + tt]
                                nc.vector.tensor_copy(
                                    vt[:, :, 0:DK],
                                    ps[:, :].rearrange("p (h d) -> p h d", d=DK),
                                )

            # ------ Phase 2: q-proj + attention + FC1, fused per q-block ---
            with (
                tc.tile_pool(name="p2s", bufs=2, space="PSUM") as spool,
                tc.tile_pool(name="p2pv", bufs=2, space="PSUM") as pvpool,
                tc.tile_pool(name="p2x", bufs=2, space="PSUM") as xpool,
                tc.tile_pool(name="p2e", bufs=8) as epool,
                tc.tile_pool(name="p2r", bufs=2) as rpool,
                tc.tile_pool(name="p2c", bufs=2) as cpool,
                tc.tile_pool(name="p2q", bufs=2) as qpool,
                tc.tile_pool(name="p3w", bufs=1) as w1pool,
                tc.tile_pool(name="p3o", bufs=2) as opool,
            ):
                w1sb = w1pool.tile([128, KF, D], f32r, tag="w1")
                nc.sync.dma_start(
                    out=w1sb[:, :, :],
                    in_=w1T[:, :].rearrange("(kt p) o -> p kt o", p=128),
                )

                def qproj_m(qb, qh, xin, m):
                    ps = xpool.tile([128, 512], f32, tag="x", name="qps")
                    for kk in range(KD):
                        nc.tensor.matmul(
                            ps[:, :],
                            lhsT=qw[:, kk, ts(m, 128)],
                            rhs=xin[kk][:, :],
                            start=(kk == 0),
                            stop=(kk == KD - 1),
                        )
                    nc.vector.tensor_scalar_add(
                        qh[m][:, :], ps[:, :], qbias[:, m : m + 1]
                    )

                def q_load(qb):
                    qh = [
                        qpool.tile([128, 512], bf16, tag=f"qh{m}", name=f"qh{m}")
                        for m in range(MT)
                    ]
                    xin = []
                    for kk in range(KD):
                        xt = inpool.tile([128, 512], bf16, tag="xin")
                        nc.sync.dma_start(
                            out=xt[:, :], in_=qT[ts(kk, 128), ts(qb, 512)]
                        )
                        xin.append(xt)
                    return qh, xin

                def attn_chunk(qh, h, pv, ktp):
                    hp, h01 = divmod(h, 2)
                    po = h01 * 64
                    sps = spool.tile([128, 1024], f32, tag="s", name="sps")
                    for j in range(2):
                        kt = 2 * ktp + j
                        nc.tensor.matmul(
                            sps[:, ts(j, 512)],
                            lhsT=khT[hp][po : po + 64, ts(kt, 128)],
                            rhs=qh[hp][po : po + 64, :],
                            start=True,
                            stop=True,
                        )
                    e = epool.tile([128, 1024], bf16, tag="e", name="e")
                    nc.scalar.activation(
                        e[:, :], sps[:, :], Exp,
                        bias=cbias[:, 0:1], scale=SCALE,
                    )
                    for j in range(2):
                        kt = 2 * ktp + j
                        nc.tensor.matmul(
                            pv[:, :],
                            lhsT=vh[kt][:, h, :],
                            rhs=e[:, ts(j, 512)],
                            start=(kt == 0),
                            stop=(kt == KT - 1),
                        )

                def attn_norm(ctx, h, pv):
                    hp, h01 = divmod(h, 2)
                    po = h01 * 64
                    za = rpool.tile([1, 512], f32r, tag="za", name="za")
                    nc.vector.tensor_copy(za[0:1, :], pv[64:65, :])
                    zbps = xpool.tile([128, 512], f32, tag="x", name="zbps")
                    nc.tensor.matmul(
                        zbps[0:64, :], lhsT=ones64[:, :], rhs=za[:, :],
                        start=True, stop=True,
                    )
                    rrec = rpool.tile([64, 512], f32, tag="rrec", name="rrec")
                    nc.vector.reciprocal(rrec[:, :], zbps[0:64, :])
                    nc.vector.tensor_mul(
                        ctx[hp][po : po + 64, :], pv[0:64, :], rrec[0:64, :]
                    )

                def fc1_tq(ctx, tq, qt):
                    osb = opool.tile([128, D], f32, tag="o")
                    for nb in range(NB):
                        fps = xpool.tile([128, 512], f32, tag="x", name="fps")
                        for kt4 in range(KF):
                            nc.tensor.matmul(
                                fps[:, :],
                                lhsT=ctx[kt4][:, ts(tq, 128)],
                                rhs=w1sb[:, kt4, ts(nb, 512)],
                                start=(kt4 == 0),
                                stop=(kt4 == KF - 1),
                            )
                        nc.vector.tensor_copy(osb[:, ts(nb, 512)], fps[:, :])
                    nc.sync.dma_start(out=out[ts(qt, 128), :], in_=osb[:, :])

                prev = None     # (ctx, qb) of the previous block, for FC1
                pending = None  # deferred normalization of the last head
                qh, xin = q_load(0)
                for m in range(MT):
                    qproj_m(0, qh, xin, m)
                for qb in range(TB):
                    ctx = [
                        cpool.tile([128, 512], f32r, tag=f"ctx{hp}", name=f"ctx{hp}")
                        for hp in range(MT)
                    ]
                    if qb + 1 < TB:
                        qh_next, xin_next = q_load(qb + 1)
                    for h in range(H_LOC):
                        pv = pvpool.tile([65, 512], f32, tag="pv", name="pv")
                        for ktp in range(KT // 2):
                            attn_chunk(qh, h, pv, ktp)
                        if pending is not None:
                            attn_norm(*pending)
                        pending = (ctx, h, pv)
                        # PE gap fillers: FC1 of the previous block and the
                        # next block's q-projection, one chunk per head.
                        if h % 2 == 0:
                            if prev is not None:
                                fc1_tq(prev[0], h // 2, prev[1] * 4 + h // 2)
                        else:
                            if qb + 1 < TB:
                                qproj_m(qb + 1, qh_next, xin_next, h // 2)
                    prev = (ctx, qb)
                    if qb + 1 < TB:
                        qh, xin = qh_next, xin_next
                attn_norm(*pending)
                for tq in range(4):
                    fc1_tq(prev[0], tq, prev[1] * 4 + tq)

    nc.compile()
    return nc


_module_cache = {}


def _get_module(seq_len=L):
    if seq_len not in _module_cache:
        _module_cache[seq_len] = build_module(seq_len)
    return _module_cache[seq_len]


def make_in_maps(q, k_s, v_s, wq, bq, wk, bk, wv, bv, w_fc1, b_fc1):
    """Host-side sharding: per-core input dict (transposed layouts)."""
    import ml_dtypes
    bf = ml_dtypes.bfloat16
    in_maps = []
    for c in range(N_CORES):
        b, g = divmod(c, N_GROUPS)
        sl = slice(g * DOUT, (g + 1) * DOUT)
        in_maps.append(
            {
                "qT": np.ascontiguousarray(q[b].T).astype(bf),
                "kT": np.ascontiguousarray(k_s[b].T).astype(bf),
                "vT": np.ascontiguousarray(v_s[b].T).astype(bf),
                "wqT": np.ascontiguousarray(wq[sl, :].T).astype(bf),
                "wkT": np.ascontiguousarray(wk[sl, :].T).astype(bf),
                "wvT": np.ascontiguousarray(wv[sl, :].T).astype(bf),
                "w1T": np.ascontiguousarray(w_fc1[:, sl].T),
                "bq2": np.ascontiguousarray(bq[sl].reshape(DOUT // 128, 128).T),
                "bk2": np.ascontiguousarray(bk[sl].reshape(DOUT // 128, 128).T),
            }
        )
    return in_maps


def kernel(q, k_w, v_w, k_s, v_s, wq, bq, wk, bk, wv, bv, w_fc1, b_fc1):
    from concourse.bass_utils import run_bass_kernel_spmd

    q = np.asarray(q, np.float32)
    k_s = np.asarray(k_s, np.float32)
    v_s = np.asarray(v_s, np.float32)
    wq = np.asarray(wq, np.float32)
    bq = np.asarray(bq, np.float32)
    wk = np.asarray(wk, np.float32)
    bk = np.asarray(bk, np.float32)
    wv = np.asarray(wv, np.float32)
    bv = np.asarray(bv, np.float32)
    w_fc1 = np.asarray(w_fc1, np.float32)
    b_fc1 = np.asarray(b_fc1, np.float32)

    nc = _get_module(L)
    in_maps = make_in_maps(q, k_s, v_s, wq, bq, wk, bk, wv, bv, w_fc1, b_fc1)
    res = run_bass_kernel_spmd(nc, in_maps, core_ids=list(range(N_CORES)))

    # Gather: sum the two row-parallel FC1 partials per batch, then add the
    # affine terms folded out of the device kernel (exact):
    #   ctx = P@V + bv  =>  out += bv @ w_fc1.T ; plus b_fc1.
    const = (b_fc1 + bv @ w_fc1.T).astype(np.float32)
    out = np.empty((B, L, D), np.float32)
    for b in range(B):
        out[b] = (
            res.results[N_GROUPS * b]["out"]
            + res.results[N_GROUPS * b + 1]["out"]
            + const
        )
    return out
